# revision 18
# baseline (speedup 1.0000x reference)
"""Bior 2x upsampling (zero-interleave + separable 9-tap filter) on 8 TRN2 cores.

Math: y[n] = sum_m h[n+4-2m] x[m] along each spatial axis (SAME zero padding).
Both separable stages are banded matmuls on the TensorEngine:

  stage 1: T1[w, nh] = sum_h X[h, w]  * A[nh, h]   (lhsT = X,  K = h)
  stage 2: Y[nh, nw] = sum_w T1[w, nh] * A[nw, w]  (lhsT = T1, K = w)

with A[n, m] = h[n+4-2m].  K-tile t (m in [128t, 128t+128)) contributes to
outputs n in [256t-4, 256t+259).  PSUM bank t accumulates out cols
[256t, 256t+256) EXACTLY (256-aligned), via one main [K=128, M=128, N=256]
matmul (rhs = C[:, 4:260] of the single constant C[i, l] = h[l - 2i],
[128, 264]); the three 8-col K-tile straddles are resolved ON the PE by six
tiny N=4 accumulate-matmuls (see _emit_block), so the four banks come out
fully resolved and evacuation is just TWO strided pair-copies per block
(ACT banks 0+1 -> out[0:512), DVE banks 2+3 -> out[512:1024)).  GPSIMD
cannot read PSUM and TensorTensor allows only one PSUM operand, which is
what pushes the straddle resolution onto the PE.

Everything is fp16 (input, consts, T1, output): matmuls run at 1 row/cycle
(same as f32r) but HBM traffic halves to 1 MiB in + 4 MiB out per core
(the rel-err budget 2e-2 >> fp16's ~1.5e-3).  Host up/down-converts f32.

Sharding: pure data parallel, 2 images per core across 8 cores.
"""

import numpy as np

H_TILDE = np.array([0.03782845550699535, -0.02384946501937986, -0.1106244044184226,
                    0.3774028556126536, 0.8526986790094022, 0.3774028556126537,
                    -0.1106244044184226, -0.02384946501937986, 0.03782845550699535],
                   dtype=np.float32)

B_PER_CORE = 2
N_CORES = 8
H = W = 512
HO = WO = 1024

STAGGERED = False
TINY = True
MM_DTYPE = "f16"   # f16 | bf16 | f32r  (data/const/t1 dtype fed to the PE)
Y_GROUP = 4
PSP_BUFS = 4
XP_BUFS = 3
T1P_BUFS = 24
YP_BUFS = 8
EVAC_ONLY = None    # None: ACT lo / DVE hi; "act"/"dve": one engine does both
IN_ENG = "scalar"   # queue for the input DMA
OUT_ENG = "sync"    # "sync" | "split" (split = yhi on gpsimd SWDGE; broken in For_i)

_CACHE = {}


def _consts():
    """C[i, l] = h[l - 2i] for 0 <= l-2i <= 8, [128, 264] fp16.

    rhs slices: bank 0 uses C[:, 4:264] (cols l<4 are n<0, clipped);
    banks 1, 2 use C[:, 0:264]; bank 3 uses C[:, 0:260] (n>=1024 clipped)."""
    c = np.zeros((128, 264), dtype=np.float32)
    for i in range(128):
        for l in range(2 * i, min(264, 2 * i + 9)):
            c[i, l] = H_TILDE[l - 2 * i]
    return c.astype(np.float16)


def _split_multiwaits(nc, mybir):
    """walrus here encodes at most ONE sem-wait per instruction; hoist extras
    onto preceding same-engine nops (sequencer order => identical semantics)."""
    ctr = 0
    for fn in nc.m.functions:
        for bb in fn.blocks:
            out, changed = [], False
            for ins in bb.instructions:
                si = ins.sync_info
                if si is not None and len(si.on_wait) > 1:
                    waits = list(si.on_wait)
                    for w in waits[:-1]:
                        ctr += 1
                        nop = mybir.InstNoOp(name=f"wsplit-{ctr}", ins=[], outs=[])
                        nop.engine = ins.engine
                        nop.sync_info = mybir.SyncInfo(on_wait=[w], on_update=[])
                        out.append(nop)
                    si.on_wait = [waits[-1]]
                    changed = True
                out.append(ins)
            if changed:
                bb.instructions = out
    return ctr


def _emit_block(nc, pa, pb, src, mlo, mhi, c):
    """One 128-row output block, split into two PSUM tiles: pa [128,2,512]
    holds out cols [0,512) (banks 0-1), pb holds [512,1024) (banks 2-3).
    Bank t covers out cols [256t, 256t+256) exactly; 4 main N=256 matmuls +
    6 tiny N=4 accumulate-matmuls resolve the three straddles on the PE.
    MMs are grouped by lhsT (one stationary load each); the first
    chronological writer of each bank carries start=True (has_written:
    clears bank, writes+sets; later writers accumulate where set,
    overwrite where not)."""
    mm = nc.tensor.matmul
    kw = dict(skip_group_check=True)
    s = [t[:, mlo:mhi] for t in src]
    main, lo, hi = c[:, 4:260], c[:, 0:4], c[:, 260:264]
    if not TINY:  # timing ablation only: straddle cols stay unresolved
        for t, (ps, k) in enumerate([(pa, 0), (pa, 1), (pb, 0), (pb, 1)]):
            mm(ps[:, k, 0:256], lhsT=s[t], rhs=main, start=True, stop=True, **kw)
        return
    mm(pa[:, 0, 0:256], lhsT=s[0], rhs=main, start=True, stop=False, **kw)   # M0
    mm(pa[:, 1, 0:4], lhsT=s[0], rhs=hi, start=True, stop=False, **kw)       # T0->b1
    mm(pa[:, 1, 0:256], lhsT=s[1], rhs=main, start=False, stop=False, **kw)  # M1
    mm(pa[:, 0, 252:256], lhsT=s[1], rhs=lo, start=False, stop=True, **kw)   # T1->b0
    mm(pb[:, 0, 0:4], lhsT=s[1], rhs=hi, start=True, stop=False, **kw)       # T1->b2
    mm(pb[:, 0, 0:256], lhsT=s[2], rhs=main, start=False, stop=False, **kw)  # M2
    mm(pa[:, 1, 252:256], lhsT=s[2], rhs=lo, start=False, stop=True, **kw)   # T2->b1
    mm(pb[:, 1, 0:4], lhsT=s[2], rhs=hi, start=True, stop=False, **kw)       # T2->b3
    mm(pb[:, 1, 0:256], lhsT=s[3], rhs=main, start=False, stop=True, **kw)   # M3
    mm(pb[:, 0, 252:256], lhsT=s[3], rhs=lo, start=False, stop=True, **kw)   # T3->b2


def _evac_half(nc, eng, ps, out):
    """ps [128, 2, 512] f32 -> out [128, 512] fp16, one strided pair-copy."""
    o = out.rearrange("p (k c) -> p k c", c=256)
    if eng == "act":
        nc.scalar.copy(out=o, in_=ps[:, :, 0:256])
    else:
        nc.vector.tensor_copy(out=o, in_=ps[:, :, 0:256])


def _build_program(reps=1, timing_mode=False, loop_n=None,
                   skip_in=False, skip_out=False, skip_compute=False,
                   skip_mm=False, skip_evac=False):
    import concourse.bass as bass
    import concourse.mybir as mybir
    import concourse.tile as tile

    f32 = mybir.dt.float32
    f16 = {"f16": mybir.dt.float16, "bf16": mybir.dt.bfloat16,
           "f32r": mybir.dt.float32r}[MM_DTYPE]

    nc = bass.Bass("TRN2", target_bir_lowering=False, debug=False,
                   num_devices=N_CORES)
    if timing_mode:
        # same dataflow, but keep the big tensors device-internal so the
        # per-call wall isn't dominated by host<->device shipping
        x_d = nc.dram_tensor("x", [B_PER_CORE, H, W], f16, kind="Internal")
        y_d = nc.dram_tensor("y", [B_PER_CORE, HO, WO], f16, kind="Internal")
        ydummy_d = nc.dram_tensor("ydummy", [1, 4], f32, kind="ExternalOutput")
    else:
        x_d = nc.dram_tensor("x", [B_PER_CORE, H, W], f16, kind="ExternalInput")
        y_d = nc.dram_tensor("y", [B_PER_CORE, HO, WO], f16, kind="ExternalOutput")
    c_d = nc.dram_tensor("c", [128, 264], f16, kind="ExternalInput")

    with tile.TileContext(nc) as tc:
        with tc.tile_pool(name="consts", bufs=1) as constp, \
             tc.tile_pool(name="xp", bufs=XP_BUFS) as xp, \
             tc.tile_pool(name="t1p", bufs=T1P_BUFS) as t1p, \
             tc.tile_pool(name="yp", bufs=YP_BUFS) as yp, \
             tc.tile_pool(name="psp", bufs=PSP_BUFS, space="PSUM") as psp:

            c_t = constp.tile([128, 264], f16)
            nc.scalar.dma_start(out=c_t[:], in_=c_d.ap())
            if timing_mode:
                nel = 16 // mybir.dt.size(f16)
                nc.sync.dma_start(out=ydummy_d.ap(),
                                  in_=c_t[0:1, 0:nel].bitcast(f32))

            def body():
                # single 1MB input DMA: one read burst instead of several cuts
                # HBM read/write turnaround
                x_big = xp.tile([128, B_PER_CORE * 4, W], f16, tag="x",
                                name="x_big")
                if not skip_in:
                    eng = {"scalar": nc.scalar, "vector": nc.vector,
                           "sync": nc.sync, "gpsimd": nc.gpsimd}[IN_ENG]
                    eng.dma_start(
                        out=x_big[:],
                        in_=x_d.ap().rearrange("b (t p) w -> p (b t) w", p=128))
                else:
                    nc.gpsimd.memset(x_big[:, :, 0:16].bitcast(f32), 0.0)

                def block(srcs, mlo, mhi, out_lo, out_hi, nm):
                    pa = psp.tile([128, 2, 512], f32, tag="ps", name=f"pa{nm}")
                    pb = psp.tile([128, 2, 512], f32, tag="ps", name=f"pb{nm}")
                    if not skip_mm:
                        _emit_block(nc, pa, pb, srcs, mlo, mhi, c_t)
                    else:
                        for ps in (pa, pb):
                            nc.tensor.matmul(ps[:, 0, 0:4], lhsT=srcs[0][:, 0:128],
                                             rhs=c_t[:, 0:4], start=True,
                                             stop=True, skip_group_check=True)
                    if not skip_evac:
                        if EVAC_ONLY is None:
                            _evac_half(nc, "act", pa, out_lo)
                            _evac_half(nc, "dve", pb, out_hi)
                        else:
                            _evac_half(nc, EVAC_ONLY, pa, out_lo)
                            _evac_half(nc, EVAC_ONLY, pb, out_hi)
                    else:
                        nc.gpsimd.memset(out_lo[:, 0:16].bitcast(f32), 0.0)
                        nc.gpsimd.memset(out_hi[:, 0:16].bitcast(f32), 0.0)

                for b in range(B_PER_CORE):
                    xt = [x_big[:, 4 * b + t, :] for t in range(4)]

                    t1lo, t1hi = [], []
                    for m in range(4):
                        tl = t1p.tile([128, 512], f16, tag="t1lo",
                                      name=f"t1lo_{b}_{m}")
                        th = t1p.tile([128, 512], f16, tag="t1hi",
                                      name=f"t1hi_{b}_{m}")
                        if not skip_compute:
                            block(xt, 128 * m, 128 * (m + 1), tl, th,
                                  f"1_{b}_{m}")
                        else:
                            nc.gpsimd.memset(tl[:, 0:16].bitcast(f32), 0.0)
                            nc.gpsimd.memset(th[:, 0:16].bitcast(f32), 0.0)
                        t1lo.append(tl)
                        t1hi.append(th)

                    for rp in range(8 // Y_GROUP):
                        ylo = yp.tile([128, Y_GROUP, 512], f16, tag="ylo",
                                      name=f"ylo_{b}_{rp}")
                        yhi = yp.tile([128, Y_GROUP, 512], f16, tag="yhi",
                                      name=f"yhi_{b}_{rp}")
                        for j in range(Y_GROUP):
                            r = Y_GROUP * rp + j
                            t1 = t1lo if r < 4 else t1hi
                            mlo = 128 * r if r < 4 else 128 * (r - 4)
                            if not skip_compute:
                                block(t1, mlo, mlo + 128,
                                      ylo[:, j, :], yhi[:, j, :],
                                      f"2_{b}_{r}")
                            else:
                                nc.gpsimd.memset(ylo[:, j, 0:16].bitcast(f32), 0.0)
                                nc.gpsimd.memset(yhi[:, j, 0:16].bitcast(f32), 0.0)
                        if not skip_out:
                            eng2 = nc.gpsimd if OUT_ENG == "split" else nc.sync
                            ydst = y_d.ap()[b].rearrange("(r p) c -> p r c", p=128)
                            rs = slice(Y_GROUP * rp, Y_GROUP * (rp + 1))
                            nc.sync.dma_start(out=ydst[:, rs, 0:512], in_=ylo[:])
                            eng2.dma_start(out=ydst[:, rs, 512:1024], in_=yhi[:])

            if loop_n is not None:
                with tc.For_i(0, loop_n, 1, staggered_reset=STAGGERED):
                    body()
            else:
                for _ in range(reps):
                    body()

    _split_multiwaits(nc, mybir)
    return nc


def _get_program():
    if "nc" not in _CACHE:
        _CACHE["nc"] = _build_program()
        _CACHE["c"] = _consts()
    return _CACHE["nc"], _CACHE["c"]


def kernel(image_batch: np.ndarray) -> np.ndarray:
    from concourse.bass_utils import run_bass_kernel_spmd

    nc, c = _get_program()
    x = np.ascontiguousarray(
        np.asarray(image_batch, dtype=np.float32).reshape(16, H, W)
        .astype(np.float16))
    in_maps = [
        {"x": x[B_PER_CORE * k:B_PER_CORE * (k + 1)], "c": c}
        for k in range(N_CORES)
    ]
    res = run_bass_kernel_spmd(nc, in_maps, core_ids=list(range(N_CORES)))
    out = np.concatenate([r["y"] for r in res.results], axis=0)
    return out.astype(np.float32).reshape(16, HO, WO, 1)


# revision 20
# speedup vs baseline: 1.1647x; 1.1647x over previous
"""Bior 2x upsampling (zero-interleave + separable 9-tap filter) on 8 TRN2 cores.

Math: y[n] = sum_m h[n+4-2m] x[m] along each spatial axis (SAME zero padding).
Both separable stages are banded matmuls on the TensorEngine:

  stage 1: T1[w, nh] = sum_h X[h, w]  * A[nh, h]   (lhsT = X,  K = h)
  stage 2: Y[nh, nw] = sum_w T1[w, nh] * A[nw, w]  (lhsT = T1, K = w)

with A[n, m] = h[n+4-2m].  K-tile t (m in [128t, 128t+128)) contributes to
outputs n in [256t-4, 256t+259).  PSUM bank t accumulates out cols
[256t, 256t+256) EXACTLY (256-aligned), via one main [K=128, M=128, N=256]
matmul (rhs = C[:, 4:260] of the single constant C[i, l] = h[l - 2i],
[128, 264]); the three 8-col K-tile straddles are resolved ON the PE by six
tiny N=4 accumulate-matmuls (see _emit_block), so the four banks come out
fully resolved and evacuation is just TWO strided pair-copies per block
(ACT banks 0+1 -> out[0:512), DVE banks 2+3 -> out[512:1024)).  GPSIMD
cannot read PSUM and TensorTensor allows only one PSUM operand, which is
what pushes the straddle resolution onto the PE.

Everything is fp16 (input, consts, T1, output): HBM traffic halves to
1 MiB in + 4 MiB out per core (rel-err budget 2e-2 >> fp16's ~1.1e-3);
host up/down-converts f32.  Measured (axon trn2): evacuation on ACT+DVE
paces compute (~395 ns/block each, decoupled halves); PE ~4 us/iter;
DMA 5.2 MiB/iter.  Decoupling PSUM/t1/y into per-engine half tiles took
the loop from ~45 us to ~24 us/iter.

Sharding: pure data parallel, 2 images per core across 8 cores.
"""

import numpy as np

H_TILDE = np.array([0.03782845550699535, -0.02384946501937986, -0.1106244044184226,
                    0.3774028556126536, 0.8526986790094022, 0.3774028556126537,
                    -0.1106244044184226, -0.02384946501937986, 0.03782845550699535],
                   dtype=np.float32)

B_PER_CORE = 2
N_CORES = 8
H = W = 512
HO = WO = 1024

STAGGERED = False
TINY = True
MM_DTYPE = "f16"   # f16 | bf16 | f32r  (data/const/t1 dtype fed to the PE)
Y_GROUP = 2
PSP_BUFS = 4
XP_BUFS = 3
T1P_BUFS = 24
YP_BUFS = 12
EVAC_ONLY = None    # None: ACT lo / DVE hi; "act"/"dve": one engine does both
IN_ENG = "scalar"   # queue for the input DMA
OUT_ENG = "sync"    # "sync" | "split" (split = yhi on gpsimd SWDGE; broken in For_i)

_CACHE = {}


def _consts():
    """C[i, l] = h[l - 2i] for 0 <= l-2i <= 8, [128, 264] fp16.

    rhs slices: bank 0 uses C[:, 4:264] (cols l<4 are n<0, clipped);
    banks 1, 2 use C[:, 0:264]; bank 3 uses C[:, 0:260] (n>=1024 clipped)."""
    c = np.zeros((128, 264), dtype=np.float32)
    for i in range(128):
        for l in range(2 * i, min(264, 2 * i + 9)):
            c[i, l] = H_TILDE[l - 2 * i]
    return c.astype(np.float16)


def _split_multiwaits(nc, mybir):
    """walrus here encodes at most ONE sem-wait per instruction; hoist extras
    onto preceding same-engine nops (sequencer order => identical semantics)."""
    ctr = 0
    for fn in nc.m.functions:
        for bb in fn.blocks:
            out, changed = [], False
            for ins in bb.instructions:
                si = ins.sync_info
                if si is not None and len(si.on_wait) > 1:
                    waits = list(si.on_wait)
                    for w in waits[:-1]:
                        ctr += 1
                        nop = mybir.InstNoOp(name=f"wsplit-{ctr}", ins=[], outs=[])
                        nop.engine = ins.engine
                        nop.sync_info = mybir.SyncInfo(on_wait=[w], on_update=[])
                        out.append(nop)
                    si.on_wait = [waits[-1]]
                    changed = True
                out.append(ins)
            if changed:
                bb.instructions = out
    return ctr


def _emit_block(nc, pa, pb, src, mlo, mhi, c):
    """One 128-row output block, split into two PSUM tiles: pa [128,2,512]
    holds out cols [0,512) (banks 0-1), pb holds [512,1024) (banks 2-3).
    Bank t covers out cols [256t, 256t+256) exactly; 4 main N=256 matmuls +
    6 tiny N=4 accumulate-matmuls resolve the three straddles on the PE.
    MMs are grouped by lhsT (one stationary load each); the first
    chronological writer of each bank carries start=True (has_written:
    clears bank, writes+sets; later writers accumulate where set,
    overwrite where not)."""
    mm = nc.tensor.matmul
    kw = dict(skip_group_check=True)
    s = [t[:, mlo:mhi] for t in src]
    main, lo, hi = c[:, 4:260], c[:, 0:4], c[:, 260:264]
    if not TINY:  # timing ablation only: straddle cols stay unresolved
        for t, (ps, k) in enumerate([(pa, 0), (pa, 1), (pb, 0), (pb, 1)]):
            mm(ps[:, k, 0:256], lhsT=s[t], rhs=main, start=True, stop=True, **kw)
        return
    mm(pa[:, 0, 0:256], lhsT=s[0], rhs=main, start=True, stop=False, **kw)   # M0
    mm(pa[:, 1, 0:4], lhsT=s[0], rhs=hi, start=True, stop=False, **kw)       # T0->b1
    mm(pa[:, 1, 0:256], lhsT=s[1], rhs=main, start=False, stop=False, **kw)  # M1
    mm(pa[:, 0, 252:256], lhsT=s[1], rhs=lo, start=False, stop=True, **kw)   # T1->b0
    mm(pb[:, 0, 0:4], lhsT=s[1], rhs=hi, start=True, stop=False, **kw)       # T1->b2
    mm(pb[:, 0, 0:256], lhsT=s[2], rhs=main, start=False, stop=False, **kw)  # M2
    mm(pa[:, 1, 252:256], lhsT=s[2], rhs=lo, start=False, stop=True, **kw)   # T2->b1
    mm(pb[:, 1, 0:4], lhsT=s[2], rhs=hi, start=True, stop=False, **kw)       # T2->b3
    mm(pb[:, 1, 0:256], lhsT=s[3], rhs=main, start=False, stop=True, **kw)   # M3
    mm(pb[:, 0, 252:256], lhsT=s[3], rhs=lo, start=False, stop=True, **kw)   # T3->b2


def _evac_half(nc, eng, ps, out):
    """ps [128, 2, 512] f32 -> out [128, 512] fp16, one strided pair-copy."""
    o = out.rearrange("p (k c) -> p k c", c=256)
    if eng == "act":
        nc.scalar.copy(out=o, in_=ps[:, :, 0:256])
    else:
        nc.vector.tensor_copy(out=o, in_=ps[:, :, 0:256])


def _build_program(reps=1, timing_mode=False, loop_n=None,
                   skip_in=False, skip_out=False, skip_compute=False,
                   skip_mm=False, skip_evac=False):
    import concourse.bass as bass
    import concourse.mybir as mybir
    import concourse.tile as tile

    f32 = mybir.dt.float32
    f16 = {"f16": mybir.dt.float16, "bf16": mybir.dt.bfloat16,
           "f32r": mybir.dt.float32r}[MM_DTYPE]

    nc = bass.Bass("TRN2", target_bir_lowering=False, debug=False,
                   num_devices=N_CORES)
    if timing_mode:
        # same dataflow, but keep the big tensors device-internal so the
        # per-call wall isn't dominated by host<->device shipping
        x_d = nc.dram_tensor("x", [B_PER_CORE, H, W], f16, kind="Internal")
        y_d = nc.dram_tensor("y", [B_PER_CORE, HO, WO], f16, kind="Internal")
        ydummy_d = nc.dram_tensor("ydummy", [1, 4], f32, kind="ExternalOutput")
    else:
        x_d = nc.dram_tensor("x", [B_PER_CORE, H, W], f16, kind="ExternalInput")
        y_d = nc.dram_tensor("y", [B_PER_CORE, HO, WO], f16, kind="ExternalOutput")
    c_d = nc.dram_tensor("c", [128, 264], f16, kind="ExternalInput")

    with tile.TileContext(nc) as tc:
        with tc.tile_pool(name="consts", bufs=1) as constp, \
             tc.tile_pool(name="xp", bufs=XP_BUFS) as xp, \
             tc.tile_pool(name="t1p", bufs=T1P_BUFS) as t1p, \
             tc.tile_pool(name="yp", bufs=YP_BUFS) as yp, \
             tc.tile_pool(name="psp", bufs=PSP_BUFS, space="PSUM") as psp:

            c_t = constp.tile([128, 264], f16)
            nc.scalar.dma_start(out=c_t[:], in_=c_d.ap())
            if timing_mode:
                nel = 16 // mybir.dt.size(f16)
                nc.sync.dma_start(out=ydummy_d.ap(),
                                  in_=c_t[0:1, 0:nel].bitcast(f32))

            def body():
                # single 1MB input DMA: one read burst instead of several cuts
                # HBM read/write turnaround
                x_big = xp.tile([128, B_PER_CORE * 4, W], f16, tag="x",
                                name="x_big")
                if not skip_in:
                    eng = {"scalar": nc.scalar, "vector": nc.vector,
                           "sync": nc.sync, "gpsimd": nc.gpsimd}[IN_ENG]
                    eng.dma_start(
                        out=x_big[:],
                        in_=x_d.ap().rearrange("b (t p) w -> p (b t) w", p=128))
                else:
                    nc.gpsimd.memset(x_big[:, :, 0:16].bitcast(f32), 0.0)

                def block(srcs, mlo, mhi, out_lo, out_hi, nm):
                    pa = psp.tile([128, 2, 512], f32, tag="ps", name=f"pa{nm}")
                    pb = psp.tile([128, 2, 512], f32, tag="ps", name=f"pb{nm}")
                    if not skip_mm:
                        _emit_block(nc, pa, pb, srcs, mlo, mhi, c_t)
                    else:
                        for ps in (pa, pb):
                            nc.tensor.matmul(ps[:, 0, 0:4], lhsT=srcs[0][:, 0:128],
                                             rhs=c_t[:, 0:4], start=True,
                                             stop=True, skip_group_check=True)
                    if not skip_evac:
                        if EVAC_ONLY is None:
                            _evac_half(nc, "act", pa, out_lo)
                            _evac_half(nc, "dve", pb, out_hi)
                        else:
                            _evac_half(nc, EVAC_ONLY, pa, out_lo)
                            _evac_half(nc, EVAC_ONLY, pb, out_hi)
                    else:
                        nc.gpsimd.memset(out_lo[:, 0:16].bitcast(f32), 0.0)
                        nc.gpsimd.memset(out_hi[:, 0:16].bitcast(f32), 0.0)

                for b in range(B_PER_CORE):
                    xt = [x_big[:, 4 * b + t, :] for t in range(4)]

                    t1lo, t1hi = [], []
                    for m in range(4):
                        tl = t1p.tile([128, 512], f16, tag="t1lo",
                                      name=f"t1lo_{b}_{m}")
                        th = t1p.tile([128, 512], f16, tag="t1hi",
                                      name=f"t1hi_{b}_{m}")
                        if not skip_compute:
                            block(xt, 128 * m, 128 * (m + 1), tl, th,
                                  f"1_{b}_{m}")
                        else:
                            nc.gpsimd.memset(tl[:, 0:16].bitcast(f32), 0.0)
                            nc.gpsimd.memset(th[:, 0:16].bitcast(f32), 0.0)
                        t1lo.append(tl)
                        t1hi.append(th)

                    for rp in range(8 // Y_GROUP):
                        ylo = yp.tile([128, Y_GROUP, 512], f16, tag="ylo",
                                      name=f"ylo_{b}_{rp}")
                        yhi = yp.tile([128, Y_GROUP, 512], f16, tag="yhi",
                                      name=f"yhi_{b}_{rp}")
                        for j in range(Y_GROUP):
                            r = Y_GROUP * rp + j
                            t1 = t1lo if r < 4 else t1hi
                            mlo = 128 * r if r < 4 else 128 * (r - 4)
                            if not skip_compute:
                                block(t1, mlo, mlo + 128,
                                      ylo[:, j, :], yhi[:, j, :],
                                      f"2_{b}_{r}")
                            else:
                                nc.gpsimd.memset(ylo[:, j, 0:16].bitcast(f32), 0.0)
                                nc.gpsimd.memset(yhi[:, j, 0:16].bitcast(f32), 0.0)
                        if not skip_out:
                            eng2 = nc.gpsimd if OUT_ENG == "split" else nc.sync
                            ydst = y_d.ap()[b].rearrange("(r p) c -> p r c", p=128)
                            rs = slice(Y_GROUP * rp, Y_GROUP * (rp + 1))
                            nc.sync.dma_start(out=ydst[:, rs, 0:512], in_=ylo[:])
                            eng2.dma_start(out=ydst[:, rs, 512:1024], in_=yhi[:])

            if loop_n is not None:
                with tc.For_i(0, loop_n, 1, staggered_reset=STAGGERED):
                    body()
            else:
                for _ in range(reps):
                    body()

    _split_multiwaits(nc, mybir)
    return nc


def _get_program():
    if "nc" not in _CACHE:
        _CACHE["nc"] = _build_program()
        _CACHE["c"] = _consts()
    return _CACHE["nc"], _CACHE["c"]


def kernel(image_batch: np.ndarray) -> np.ndarray:
    from concourse.bass_utils import run_bass_kernel_spmd

    nc, c = _get_program()
    x = np.ascontiguousarray(
        np.asarray(image_batch, dtype=np.float32).reshape(16, H, W)
        .astype(np.float16))
    in_maps = [
        {"x": x[B_PER_CORE * k:B_PER_CORE * (k + 1)], "c": c}
        for k in range(N_CORES)
    ]
    res = run_bass_kernel_spmd(nc, in_maps, core_ids=list(range(N_CORES)))
    out = np.concatenate([r["y"] for r in res.results], axis=0)
    return out.astype(np.float32).reshape(16, HO, WO, 1)


# revision 24
# speedup vs baseline: 1.5927x; 1.3675x over previous
"""Bior 2x upsampling (zero-interleave + separable 9-tap filter) on 8 TRN2 cores.

Math: y[n] = sum_m h[n+4-2m] x[m] along each spatial axis (SAME zero padding).
Both separable stages are banded matmuls on the TensorEngine:

  stage 1: T1[w, nh] = sum_h X[h, w]  * A[nh, h]   (lhsT = X,  K = h)
  stage 2: Y[nh, nw] = sum_w T1[w, nh] * A[nw, w]  (lhsT = T1, K = w)

with A[n, m] = h[n+4-2m].  K-tile t (m in [128t, 128t+128)) contributes to
outputs n in [256t-4, 256t+259).  PSUM bank t accumulates out cols
[256t, 256t+256) EXACTLY (256-aligned), via one main [K=128, M=128, N=256]
matmul (rhs = C[:, 4:260] of the single constant C[i, l] = h[l - 2i],
[128, 264]); the three 8-col K-tile straddles are resolved ON the PE by six
tiny N=4 accumulate-matmuls (see _emit_block), so the four banks come out
fully resolved and evacuation is just TWO strided pair-copies per block
(ACT banks 0+1 -> out[0:512), DVE banks 2+3 -> out[512:1024)).  GPSIMD
cannot read PSUM and TensorTensor allows only one PSUM operand, which is
what pushes the straddle resolution onto the PE.

Everything is fp16 (input, consts, T1, output): HBM traffic halves to
1 MiB in + 4 MiB out per core (rel-err budget 2e-2 >> fp16's ~1.1e-3);
host up/down-converts f32.  Measured (axon trn2): evacuation on ACT+DVE
paces compute (~395 ns/block each, decoupled halves); PE ~4 us/iter;
DMA 5.2 MiB/iter.  Decoupling PSUM/t1/y into per-engine half tiles took
the loop from ~45 us to ~24 us/iter; unrolling the timing loop body 8x
inside For_i (BODY_UNROLL, amortizing the per-iteration all-engine
barrier) took it to ~12 us/iter.

Sharding: pure data parallel, 2 images per core across 8 cores.
"""

import numpy as np

H_TILDE = np.array([0.03782845550699535, -0.02384946501937986, -0.1106244044184226,
                    0.3774028556126536, 0.8526986790094022, 0.3774028556126537,
                    -0.1106244044184226, -0.02384946501937986, 0.03782845550699535],
                   dtype=np.float32)

B_PER_CORE = 2
N_CORES = 8
H = W = 512
HO = WO = 1024

STAGGERED = False
BODY_UNROLL = 8
TINY = True
MM_DTYPE = "f16"   # f16 | bf16 | f32r  (data/const/t1 dtype fed to the PE)
Y_GROUP = 2
PSP_BUFS = 4
XP_BUFS = 3
T1P_BUFS = 24
YP_BUFS = 12
EVAC_ONLY = None    # None: ACT lo / DVE hi; "act"/"dve": one engine does both
IN_ENG = "scalar"   # queue for the input DMA
OUT_ENG = "sync"    # "sync" | "split" (split = yhi on gpsimd SWDGE; broken in For_i)

_CACHE = {}


def _consts():
    """C[i, l] = h[l - 2i] for 0 <= l-2i <= 8, [128, 264] fp16.

    rhs slices: bank 0 uses C[:, 4:264] (cols l<4 are n<0, clipped);
    banks 1, 2 use C[:, 0:264]; bank 3 uses C[:, 0:260] (n>=1024 clipped)."""
    c = np.zeros((128, 264), dtype=np.float32)
    for i in range(128):
        for l in range(2 * i, min(264, 2 * i + 9)):
            c[i, l] = H_TILDE[l - 2 * i]
    return c.astype(np.float16)


def _split_multiwaits(nc, mybir):
    """walrus here encodes at most ONE sem-wait per instruction; hoist extras
    onto preceding same-engine nops (sequencer order => identical semantics)."""
    ctr = 0
    for fn in nc.m.functions:
        for bb in fn.blocks:
            out, changed = [], False
            for ins in bb.instructions:
                si = ins.sync_info
                if si is not None and len(si.on_wait) > 1:
                    waits = list(si.on_wait)
                    for w in waits[:-1]:
                        ctr += 1
                        nop = mybir.InstNoOp(name=f"wsplit-{ctr}", ins=[], outs=[])
                        nop.engine = ins.engine
                        nop.sync_info = mybir.SyncInfo(on_wait=[w], on_update=[])
                        out.append(nop)
                    si.on_wait = [waits[-1]]
                    changed = True
                out.append(ins)
            if changed:
                bb.instructions = out
    return ctr


def _emit_block(nc, pa, pb, src, mlo, mhi, c):
    """One 128-row output block, split into two PSUM tiles: pa [128,2,512]
    holds out cols [0,512) (banks 0-1), pb holds [512,1024) (banks 2-3).
    Bank t covers out cols [256t, 256t+256) exactly; 4 main N=256 matmuls +
    6 tiny N=4 accumulate-matmuls resolve the three straddles on the PE.
    MMs are grouped by lhsT (one stationary load each); the first
    chronological writer of each bank carries start=True (has_written:
    clears bank, writes+sets; later writers accumulate where set,
    overwrite where not)."""
    mm = nc.tensor.matmul
    kw = dict(skip_group_check=True)
    s = [t[:, mlo:mhi] for t in src]
    main, lo, hi = c[:, 4:260], c[:, 0:4], c[:, 260:264]
    if not TINY:  # timing ablation only: straddle cols stay unresolved
        for t, (ps, k) in enumerate([(pa, 0), (pa, 1), (pb, 0), (pb, 1)]):
            mm(ps[:, k, 0:256], lhsT=s[t], rhs=main, start=True, stop=True, **kw)
        return
    mm(pa[:, 0, 0:256], lhsT=s[0], rhs=main, start=True, stop=False, **kw)   # M0
    mm(pa[:, 1, 0:4], lhsT=s[0], rhs=hi, start=True, stop=False, **kw)       # T0->b1
    mm(pa[:, 1, 0:256], lhsT=s[1], rhs=main, start=False, stop=False, **kw)  # M1
    mm(pa[:, 0, 252:256], lhsT=s[1], rhs=lo, start=False, stop=True, **kw)   # T1->b0
    mm(pb[:, 0, 0:4], lhsT=s[1], rhs=hi, start=True, stop=False, **kw)       # T1->b2
    mm(pb[:, 0, 0:256], lhsT=s[2], rhs=main, start=False, stop=False, **kw)  # M2
    mm(pa[:, 1, 252:256], lhsT=s[2], rhs=lo, start=False, stop=True, **kw)   # T2->b1
    mm(pb[:, 1, 0:4], lhsT=s[2], rhs=hi, start=True, stop=False, **kw)       # T2->b3
    mm(pb[:, 1, 0:256], lhsT=s[3], rhs=main, start=False, stop=True, **kw)   # M3
    mm(pb[:, 0, 252:256], lhsT=s[3], rhs=lo, start=False, stop=True, **kw)   # T3->b2


def _evac_half(nc, eng, ps, out):
    """ps [128, 2, 512] f32 -> out [128, 512] fp16, one strided pair-copy."""
    o = out.rearrange("p (k c) -> p k c", c=256)
    if eng == "act":
        nc.scalar.copy(out=o, in_=ps[:, :, 0:256])
    else:
        nc.vector.tensor_copy(out=o, in_=ps[:, :, 0:256])


def _build_program(reps=1, timing_mode=False, loop_n=None,
                   skip_in=False, skip_out=False, skip_compute=False,
                   skip_mm=False, skip_evac=False, body_unroll=None):
    import concourse.bass as bass
    import concourse.mybir as mybir
    import concourse.tile as tile

    f32 = mybir.dt.float32
    f16 = {"f16": mybir.dt.float16, "bf16": mybir.dt.bfloat16,
           "f32r": mybir.dt.float32r}[MM_DTYPE]

    nc = bass.Bass("TRN2", target_bir_lowering=False, debug=False,
                   num_devices=N_CORES)
    if timing_mode:
        # same dataflow, but keep the big tensors device-internal so the
        # per-call wall isn't dominated by host<->device shipping
        x_d = nc.dram_tensor("x", [B_PER_CORE, H, W], f16, kind="Internal")
        y_d = nc.dram_tensor("y", [B_PER_CORE, HO, WO], f16, kind="Internal")
        ydummy_d = nc.dram_tensor("ydummy", [1, 4], f32, kind="ExternalOutput")
    else:
        x_d = nc.dram_tensor("x", [B_PER_CORE, H, W], f16, kind="ExternalInput")
        y_d = nc.dram_tensor("y", [B_PER_CORE, HO, WO], f16, kind="ExternalOutput")
    c_d = nc.dram_tensor("c", [128, 264], f16, kind="ExternalInput")

    with tile.TileContext(nc) as tc:
        with tc.tile_pool(name="consts", bufs=1) as constp, \
             tc.tile_pool(name="xp", bufs=XP_BUFS) as xp, \
             tc.tile_pool(name="t1p", bufs=T1P_BUFS) as t1p, \
             tc.tile_pool(name="yp", bufs=YP_BUFS) as yp, \
             tc.tile_pool(name="psp", bufs=PSP_BUFS, space="PSUM") as psp:

            c_t = constp.tile([128, 264], f16)
            nc.scalar.dma_start(out=c_t[:], in_=c_d.ap())
            if timing_mode:
                nel = 16 // mybir.dt.size(f16)
                nc.sync.dma_start(out=ydummy_d.ap(),
                                  in_=c_t[0:1, 0:nel].bitcast(f32))

            def body():
                # single 1MB input DMA: one read burst instead of several cuts
                # HBM read/write turnaround
                x_big = xp.tile([128, B_PER_CORE * 4, W], f16, tag="x",
                                name="x_big")
                if not skip_in:
                    eng = {"scalar": nc.scalar, "vector": nc.vector,
                           "sync": nc.sync, "gpsimd": nc.gpsimd}[IN_ENG]
                    eng.dma_start(
                        out=x_big[:],
                        in_=x_d.ap().rearrange("b (t p) w -> p (b t) w", p=128))
                else:
                    nc.gpsimd.memset(x_big[:, :, 0:16].bitcast(f32), 0.0)

                def block(srcs, mlo, mhi, out_lo, out_hi, nm):
                    pa = psp.tile([128, 2, 512], f32, tag="ps", name=f"pa{nm}")
                    pb = psp.tile([128, 2, 512], f32, tag="ps", name=f"pb{nm}")
                    if not skip_mm:
                        _emit_block(nc, pa, pb, srcs, mlo, mhi, c_t)
                    else:
                        for ps in (pa, pb):
                            nc.tensor.matmul(ps[:, 0, 0:4], lhsT=srcs[0][:, 0:128],
                                             rhs=c_t[:, 0:4], start=True,
                                             stop=True, skip_group_check=True)
                    if not skip_evac:
                        if EVAC_ONLY is None:
                            _evac_half(nc, "act", pa, out_lo)
                            _evac_half(nc, "dve", pb, out_hi)
                        else:
                            _evac_half(nc, EVAC_ONLY, pa, out_lo)
                            _evac_half(nc, EVAC_ONLY, pb, out_hi)
                    else:
                        nc.gpsimd.memset(out_lo[:, 0:16].bitcast(f32), 0.0)
                        nc.gpsimd.memset(out_hi[:, 0:16].bitcast(f32), 0.0)

                for b in range(B_PER_CORE):
                    xt = [x_big[:, 4 * b + t, :] for t in range(4)]

                    t1lo, t1hi = [], []
                    for m in range(4):
                        tl = t1p.tile([128, 512], f16, tag="t1lo",
                                      name=f"t1lo_{b}_{m}")
                        th = t1p.tile([128, 512], f16, tag="t1hi",
                                      name=f"t1hi_{b}_{m}")
                        if not skip_compute:
                            block(xt, 128 * m, 128 * (m + 1), tl, th,
                                  f"1_{b}_{m}")
                        else:
                            nc.gpsimd.memset(tl[:, 0:16].bitcast(f32), 0.0)
                            nc.gpsimd.memset(th[:, 0:16].bitcast(f32), 0.0)
                        t1lo.append(tl)
                        t1hi.append(th)

                    for rp in range(8 // Y_GROUP):
                        ylo = yp.tile([128, Y_GROUP, 512], f16, tag="ylo",
                                      name=f"ylo_{b}_{rp}")
                        yhi = yp.tile([128, Y_GROUP, 512], f16, tag="yhi",
                                      name=f"yhi_{b}_{rp}")
                        for j in range(Y_GROUP):
                            r = Y_GROUP * rp + j
                            t1 = t1lo if r < 4 else t1hi
                            mlo = 128 * r if r < 4 else 128 * (r - 4)
                            if not skip_compute:
                                block(t1, mlo, mlo + 128,
                                      ylo[:, j, :], yhi[:, j, :],
                                      f"2_{b}_{r}")
                            else:
                                nc.gpsimd.memset(ylo[:, j, 0:16].bitcast(f32), 0.0)
                                nc.gpsimd.memset(yhi[:, j, 0:16].bitcast(f32), 0.0)
                        if not skip_out:
                            eng2 = nc.gpsimd if OUT_ENG == "split" else nc.sync
                            ydst = y_d.ap()[b].rearrange("(r p) c -> p r c", p=128)
                            rs = slice(Y_GROUP * rp, Y_GROUP * (rp + 1))
                            nc.sync.dma_start(out=ydst[:, rs, 0:512], in_=ylo[:])
                            eng2.dma_start(out=ydst[:, rs, 512:1024], in_=yhi[:])

            if loop_n is not None:
                if body_unroll is None:
                    body_unroll = BODY_UNROLL
                assert loop_n % body_unroll == 0
                with tc.For_i(0, loop_n // body_unroll, 1,
                              staggered_reset=STAGGERED):
                    for _ in range(body_unroll):
                        body()
            else:
                for _ in range(reps):
                    body()

    _split_multiwaits(nc, mybir)
    return nc


def _get_program():
    if "nc" not in _CACHE:
        _CACHE["nc"] = _build_program()
        _CACHE["c"] = _consts()
    return _CACHE["nc"], _CACHE["c"]


def kernel(image_batch: np.ndarray) -> np.ndarray:
    from concourse.bass_utils import run_bass_kernel_spmd

    nc, c = _get_program()
    x = np.ascontiguousarray(
        np.asarray(image_batch, dtype=np.float32).reshape(16, H, W)
        .astype(np.float16))
    in_maps = [
        {"x": x[B_PER_CORE * k:B_PER_CORE * (k + 1)], "c": c}
        for k in range(N_CORES)
    ]
    res = run_bass_kernel_spmd(nc, in_maps, core_ids=list(range(N_CORES)))
    out = np.concatenate([r["y"] for r in res.results], axis=0)
    return out.astype(np.float32).reshape(16, HO, WO, 1)


# revision 25
# speedup vs baseline: 2.3706x; 1.4884x over previous
"""Bior 2x upsampling (zero-interleave + separable 9-tap filter) on 8 TRN2 cores.

Math: y[n] = sum_m h[n+4-2m] x[m] along each spatial axis (SAME zero padding).
Both separable stages are banded matmuls on the TensorEngine:

  stage 1: T1[w, nh] = sum_h X[h, w]  * A[nh, h]   (lhsT = X,  K = h)
  stage 2: Y[nh, nw] = sum_w T1[w, nh] * A[nw, w]  (lhsT = T1, K = w)

with A[n, m] = h[n+4-2m].  K-tile t (m in [128t, 128t+128)) contributes to
outputs n in [256t-4, 256t+259).  PSUM bank t accumulates out cols
[256t, 256t+256) EXACTLY (256-aligned), via one main [K=128, M=128, N=256]
matmul (rhs = C[:, 4:260] of the single constant C[i, l] = h[l - 2i],
[128, 264]); the three 8-col K-tile straddles are resolved ON the PE by six
tiny N=4 accumulate-matmuls (see _emit_block), so the four banks come out
fully resolved and evacuation is just TWO strided pair-copies per block
(ACT banks 0+1 -> out[0:512), DVE banks 2+3 -> out[512:1024)).  GPSIMD
cannot read PSUM and TensorTensor allows only one PSUM operand, which is
what pushes the straddle resolution onto the PE.

Everything is fp16 (input, consts, T1, output): HBM traffic halves to
1 MiB in + 4 MiB out per core (rel-err budget 2e-2 >> fp16's ~1.1e-3);
host up/down-converts f32.  Measured (axon trn2): evacuation on ACT+DVE
paces compute (~395 ns/block each, decoupled halves); PE ~4 us/iter;
DMA 5.2 MiB/iter.  Decoupling PSUM/t1/y into per-engine half tiles took
the loop from ~45 us to ~24 us/iter; unrolling the timing loop body 8x
inside For_i (BODY_UNROLL, amortizing the per-iteration all-engine
barrier) took it to ~12 us/iter.

Sharding: pure data parallel, 2 images per core across 8 cores.
"""

import numpy as np

H_TILDE = np.array([0.03782845550699535, -0.02384946501937986, -0.1106244044184226,
                    0.3774028556126536, 0.8526986790094022, 0.3774028556126537,
                    -0.1106244044184226, -0.02384946501937986, 0.03782845550699535],
                   dtype=np.float32)

B_PER_CORE = 2
N_CORES = 8
H = W = 512
HO = WO = 1024

STAGGERED = False
BODY_UNROLL = 8
TINY = True
MM_DTYPE = "f16"   # f16 | bf16 | f32r  (data/const/t1 dtype fed to the PE)
Y_GROUP = 2
PSP_BUFS = 4
XP_BUFS = 3
T1P_BUFS = 24
YP_BUFS = 12
EVAC_ONLY = None    # None: ACT lo / DVE hi; "act"/"dve": one engine does both
IN_ENG = "scalar"   # queue for the input DMA
OUT_ENG = "sync"    # "sync" | "split" (split = yhi on gpsimd SWDGE; broken in For_i)

_CACHE = {}


def _consts():
    """C[i, l] = h[l - 2i] for 0 <= l-2i <= 8, [128, 264] fp16.

    rhs slices: bank 0 uses C[:, 4:264] (cols l<4 are n<0, clipped);
    banks 1, 2 use C[:, 0:264]; bank 3 uses C[:, 0:260] (n>=1024 clipped)."""
    c = np.zeros((128, 264), dtype=np.float32)
    for i in range(128):
        for l in range(2 * i, min(264, 2 * i + 9)):
            c[i, l] = H_TILDE[l - 2 * i]
    return c.astype(np.float16)


def _split_multiwaits(nc, mybir):
    """walrus here encodes at most ONE sem-wait per instruction; hoist extras
    onto preceding same-engine nops (sequencer order => identical semantics)."""
    ctr = 0
    for fn in nc.m.functions:
        for bb in fn.blocks:
            out, changed = [], False
            for ins in bb.instructions:
                si = ins.sync_info
                if si is not None and len(si.on_wait) > 1:
                    waits = list(si.on_wait)
                    for w in waits[:-1]:
                        ctr += 1
                        nop = mybir.InstNoOp(name=f"wsplit-{ctr}", ins=[], outs=[])
                        nop.engine = ins.engine
                        nop.sync_info = mybir.SyncInfo(on_wait=[w], on_update=[])
                        out.append(nop)
                    si.on_wait = [waits[-1]]
                    changed = True
                out.append(ins)
            if changed:
                bb.instructions = out
    return ctr


def _emit_block(nc, pa, pb, src, mlo, mhi, c):
    """One 128-row output block, split into two PSUM tiles: pa [128,2,512]
    holds out cols [0,512) (banks 0-1), pb holds [512,1024) (banks 2-3).
    Bank t covers out cols [256t, 256t+256) exactly; 4 main N=256 matmuls +
    6 tiny N=4 accumulate-matmuls resolve the three straddles on the PE.
    MMs are grouped by lhsT (one stationary load each); the first
    chronological writer of each bank carries start=True (has_written:
    clears bank, writes+sets; later writers accumulate where set,
    overwrite where not)."""
    mm = nc.tensor.matmul
    kw = dict(skip_group_check=True)
    s = [t[:, mlo:mhi] for t in src]
    main, lo, hi = c[:, 4:260], c[:, 0:4], c[:, 260:264]
    if not TINY:  # timing ablation only: straddle cols stay unresolved
        for t, (ps, k) in enumerate([(pa, 0), (pa, 1), (pb, 0), (pb, 1)]):
            mm(ps[:, k, 0:256], lhsT=s[t], rhs=main, start=True, stop=True, **kw)
        return
    mm(pa[:, 0, 0:256], lhsT=s[0], rhs=main, start=True, stop=False, **kw)   # M0
    mm(pa[:, 1, 0:4], lhsT=s[0], rhs=hi, start=True, stop=False, **kw)       # T0->b1
    mm(pa[:, 1, 0:256], lhsT=s[1], rhs=main, start=False, stop=False, **kw)  # M1
    mm(pa[:, 0, 252:256], lhsT=s[1], rhs=lo, start=False, stop=True, **kw)   # T1->b0
    mm(pb[:, 0, 0:4], lhsT=s[1], rhs=hi, start=True, stop=False, **kw)       # T1->b2
    mm(pb[:, 0, 0:256], lhsT=s[2], rhs=main, start=False, stop=False, **kw)  # M2
    mm(pa[:, 1, 252:256], lhsT=s[2], rhs=lo, start=False, stop=True, **kw)   # T2->b1
    mm(pb[:, 1, 0:4], lhsT=s[2], rhs=hi, start=True, stop=False, **kw)       # T2->b3
    mm(pb[:, 1, 0:256], lhsT=s[3], rhs=main, start=False, stop=True, **kw)   # M3
    mm(pb[:, 0, 252:256], lhsT=s[3], rhs=lo, start=False, stop=True, **kw)   # T3->b2


def _evac_half(nc, eng, ps, out):
    """ps [128, 2, 512] f32 -> out [128, 512] fp16, one strided pair-copy."""
    o = out.rearrange("p (k c) -> p k c", c=256)
    if eng == "act":
        nc.scalar.copy(out=o, in_=ps[:, :, 0:256])
    else:
        nc.vector.tensor_copy(out=o, in_=ps[:, :, 0:256])


def _build_program(reps=1, timing_mode=False, loop_n=None,
                   skip_in=False, skip_out=False, skip_compute=False,
                   skip_mm=False, skip_evac=False, body_unroll=None):
    import concourse.bass as bass
    import concourse.mybir as mybir
    import concourse.tile as tile

    f32 = mybir.dt.float32
    f16 = {"f16": mybir.dt.float16, "bf16": mybir.dt.bfloat16,
           "f32r": mybir.dt.float32r}[MM_DTYPE]

    nc = bass.Bass("TRN2", target_bir_lowering=False, debug=False,
                   num_devices=N_CORES)
    if timing_mode:
        # same dataflow, but keep the big tensors device-internal so the
        # per-call wall isn't dominated by host<->device shipping
        x_d = nc.dram_tensor("x", [B_PER_CORE, H, W], f16, kind="Internal")
        y_d = nc.dram_tensor("y", [B_PER_CORE, HO, WO], f16, kind="Internal")
        ydummy_d = nc.dram_tensor("ydummy", [1, 4], f32, kind="ExternalOutput")
    else:
        x_d = nc.dram_tensor("x", [B_PER_CORE, H, W], f16, kind="ExternalInput")
        y_d = nc.dram_tensor("y", [B_PER_CORE, HO, WO], f16, kind="ExternalOutput")
    c_d = nc.dram_tensor("c", [128, 264], f16, kind="ExternalInput")

    with tile.TileContext(nc) as tc:
        with tc.tile_pool(name="consts", bufs=1) as constp, \
             tc.tile_pool(name="xp", bufs=XP_BUFS) as xp, \
             tc.tile_pool(name="t1p", bufs=T1P_BUFS) as t1p, \
             tc.tile_pool(name="yp", bufs=YP_BUFS) as yp, \
             tc.tile_pool(name="psp", bufs=PSP_BUFS, space="PSUM") as psp:

            c_t = constp.tile([128, 264], f16)
            nc.scalar.dma_start(out=c_t[:], in_=c_d.ap())
            if timing_mode:
                nel = 16 // mybir.dt.size(f16)
                nc.sync.dma_start(out=ydummy_d.ap(),
                                  in_=c_t[0:1, 0:nel].bitcast(f32))

            in_flip = [0]

            def body():
                # single 1MB input DMA: one read burst instead of several cuts
                # HBM read/write turnaround
                x_big = xp.tile([128, B_PER_CORE * 4, W], f16, tag="x",
                                name="x_big")
                if not skip_in:
                    if IN_ENG == "alt":
                        in_flip[0] += 1
                        eng = nc.scalar if in_flip[0] % 2 == 0 else nc.vector
                    else:
                        eng = {"scalar": nc.scalar, "vector": nc.vector,
                               "sync": nc.sync, "gpsimd": nc.gpsimd}[IN_ENG]
                    eng.dma_start(
                        out=x_big[:],
                        in_=x_d.ap().rearrange("b (t p) w -> p (b t) w", p=128))
                else:
                    nc.gpsimd.memset(x_big[:, :, 0:16].bitcast(f32), 0.0)

                def block(srcs, mlo, mhi, out_lo, out_hi, nm):
                    pa = psp.tile([128, 2, 512], f32, tag="ps", name=f"pa{nm}")
                    pb = psp.tile([128, 2, 512], f32, tag="ps", name=f"pb{nm}")
                    if not skip_mm:
                        _emit_block(nc, pa, pb, srcs, mlo, mhi, c_t)
                    else:
                        for ps in (pa, pb):
                            nc.tensor.matmul(ps[:, 0, 0:4], lhsT=srcs[0][:, 0:128],
                                             rhs=c_t[:, 0:4], start=True,
                                             stop=True, skip_group_check=True)
                    if not skip_evac:
                        if EVAC_ONLY is None:
                            _evac_half(nc, "act", pa, out_lo)
                            _evac_half(nc, "dve", pb, out_hi)
                        else:
                            _evac_half(nc, EVAC_ONLY, pa, out_lo)
                            _evac_half(nc, EVAC_ONLY, pb, out_hi)
                    else:
                        nc.gpsimd.memset(out_lo[:, 0:16].bitcast(f32), 0.0)
                        nc.gpsimd.memset(out_hi[:, 0:16].bitcast(f32), 0.0)

                for b in range(B_PER_CORE):
                    xt = [x_big[:, 4 * b + t, :] for t in range(4)]

                    t1lo, t1hi = [], []
                    for m in range(4):
                        tl = t1p.tile([128, 512], f16, tag="t1lo",
                                      name=f"t1lo_{b}_{m}")
                        th = t1p.tile([128, 512], f16, tag="t1hi",
                                      name=f"t1hi_{b}_{m}")
                        if not skip_compute:
                            block(xt, 128 * m, 128 * (m + 1), tl, th,
                                  f"1_{b}_{m}")
                        else:
                            nc.gpsimd.memset(tl[:, 0:16].bitcast(f32), 0.0)
                            nc.gpsimd.memset(th[:, 0:16].bitcast(f32), 0.0)
                        t1lo.append(tl)
                        t1hi.append(th)

                    for rp in range(8 // Y_GROUP):
                        ylo = yp.tile([128, Y_GROUP, 512], f16, tag="ylo",
                                      name=f"ylo_{b}_{rp}")
                        yhi = yp.tile([128, Y_GROUP, 512], f16, tag="yhi",
                                      name=f"yhi_{b}_{rp}")
                        for j in range(Y_GROUP):
                            r = Y_GROUP * rp + j
                            t1 = t1lo if r < 4 else t1hi
                            mlo = 128 * r if r < 4 else 128 * (r - 4)
                            if not skip_compute:
                                block(t1, mlo, mlo + 128,
                                      ylo[:, j, :], yhi[:, j, :],
                                      f"2_{b}_{r}")
                            else:
                                nc.gpsimd.memset(ylo[:, j, 0:16].bitcast(f32), 0.0)
                                nc.gpsimd.memset(yhi[:, j, 0:16].bitcast(f32), 0.0)
                        if not skip_out:
                            eng2 = nc.gpsimd if OUT_ENG == "split" else nc.sync
                            ydst = y_d.ap()[b].rearrange("(r p) c -> p r c", p=128)
                            rs = slice(Y_GROUP * rp, Y_GROUP * (rp + 1))
                            nc.sync.dma_start(out=ydst[:, rs, 0:512], in_=ylo[:])
                            eng2.dma_start(out=ydst[:, rs, 512:1024], in_=yhi[:])

            if loop_n is not None:
                if body_unroll is None:
                    body_unroll = BODY_UNROLL
                assert loop_n % body_unroll == 0
                with tc.For_i(0, loop_n // body_unroll, 1,
                              staggered_reset=STAGGERED):
                    for _ in range(body_unroll):
                        body()
            else:
                for _ in range(reps):
                    body()

    _split_multiwaits(nc, mybir)
    return nc


def _get_program():
    if "nc" not in _CACHE:
        _CACHE["nc"] = _build_program()
        _CACHE["c"] = _consts()
    return _CACHE["nc"], _CACHE["c"]


def kernel(image_batch: np.ndarray) -> np.ndarray:
    from concourse.bass_utils import run_bass_kernel_spmd

    nc, c = _get_program()
    x = np.ascontiguousarray(
        np.asarray(image_batch, dtype=np.float32).reshape(16, H, W)
        .astype(np.float16))
    in_maps = [
        {"x": x[B_PER_CORE * k:B_PER_CORE * (k + 1)], "c": c}
        for k in range(N_CORES)
    ]
    res = run_bass_kernel_spmd(nc, in_maps, core_ids=list(range(N_CORES)))
    out = np.concatenate([r["y"] for r in res.results], axis=0)
    return out.astype(np.float32).reshape(16, HO, WO, 1)
